# revision 7
# baseline (speedup 1.0000x reference)
"""Trainium2 Bass kernel for nn_CosineLoss (data-parallel over 8 NeuronCores).

loss = -sum_n pred[n, t[n]] / (||pred[n]|| + eps) / N
       + 0.1 * mean_n (1 - ||pred[n]||)^2

Strategy per core (8192 rows x 1000 cols, f32):
  - Stream [128, 8*1000] super-tiles from HBM (4 MB per HWDGE dma_start).
  - ACT engine: Square activation with accum_out -> per-row sum of squares.
  - GpSimd ap_gather: per 16-partition group, gather the 8 blocks' target
    columns; extract the per-partition diagonal with tiny DVE
    tensor_mul + tensor_reduce ops against a precomputed (i%16 == p%16) mask.
  - Final: sqrt / +eps / reciprocal / weighted reduces -> per-partition
    partial sums [128, 2]; host sums 8*128 partials into the scalar.
"""

import sys

for _p in ("/root/.axon_site/_ro/trn_rl_repo", "/opt/trn_rl_repo"):
    if _p not in sys.path:
        sys.path.append(_p)

import numpy as np

N = 65536
C = 1000
NCORES = 8
R = N // NCORES          # rows per core
P = 128                  # partitions
NT = R // P              # 64 row-blocks per core
SUP = 8                  # row-blocks per super-tile
NSUP = NT // SUP         # 8 super-tiles per core
EPS = 1e-9
NORM_FACTOR = 0.1

_STATE = {}


def _build_program():
    import concourse.bacc as bacc
    import concourse.bass as bass
    import concourse.mybir as mybir
    import concourse.tile as tile
    from concourse._compat import with_exitstack

    f32 = mybir.dt.float32
    i16 = mybir.dt.int16
    AF = mybir.ActivationFunctionType
    ALU = mybir.AluOpType

    nc = bacc.Bacc(
        "TRN2",
        target_bir_lowering=False,
        debug=False,
        enable_asserts=False,
        num_devices=NCORES,
    )

    pred_d = nc.dram_tensor("pred", [R, C], f32, kind="ExternalInput").ap()
    tgt_d = nc.dram_tensor("tgt", [P, NT], i16, kind="ExternalInput").ap()
    m128_d = nc.dram_tensor("m128", [P, SUP * 16], f32, kind="ExternalInput").ap()
    out_d = nc.dram_tensor("out", [P, 2], f32, kind="ExternalOutput").ap()

    # [R, C] viewed as [p, block, c]: row = block*128 + p
    pred_v = pred_d.rearrange("(sb p) c -> p sb c", p=P)

    with tile.TileContext(nc) as tc:
        from contextlib import ExitStack

        with ExitStack() as ctx:
            data_pool = ctx.enter_context(tc.tile_pool(name="data", bufs=3))
            g16_pool = ctx.enter_context(tc.tile_pool(name="g16", bufs=2))
            scr_pool = ctx.enter_context(tc.tile_pool(name="scr", bufs=2))
            junk_pool = ctx.enter_context(tc.tile_pool(name="junk", bufs=2))
            persist = ctx.enter_context(tc.tile_pool(name="persist", bufs=1))

            tgt_t = persist.tile([P, NT], i16)
            nc.sync.dma_start(tgt_t[:], tgt_d[:])
            m128_t = persist.tile([P, SUP * 16], f32)
            nc.sync.dma_start(m128_t[:], m128_d[:])

            sumsq = persist.tile([P, NT], f32)
            gath = persist.tile([P, NT], f32)

            for s in range(NSUP):
                data = data_pool.tile([P, SUP * C], f32)
                nc.sync.dma_start(data[:], pred_v[:, bass.ts(s, SUP), :])

                g16 = g16_pool.tile([P, SUP * 16], f32)
                nc.gpsimd.ap_gather(
                    g16[:],
                    data[:],
                    tgt_t[:, bass.ts(s, SUP)],
                    channels=P,
                    num_elems=SUP * C,
                    d=1,
                    num_idxs=SUP * 16,
                )

                for b in range(SUP):
                    j = s * SUP + b
                    scr = scr_pool.tile([P, C], f32)
                    nc.scalar.activation(
                        scr[:],
                        data[:, bass.ts(b, C)],
                        AF.Square,
                        accum_out=sumsq[:, j : j + 1],
                    )

                gm = junk_pool.tile([P, SUP * 16], f32)
                nc.vector.tensor_mul(gm[:], g16[:], m128_t[:])
                nc.vector.tensor_reduce(
                    gath[:, bass.ts(s, SUP)],
                    gm[:].rearrange("p (b i) -> p b i", i=16),
                    mybir.AxisListType.X,
                    ALU.add,
                )

            # Final per-partition reductions.
            norms = persist.tile([P, NT], f32)
            nc.scalar.activation(norms[:], sumsq[:], AF.Sqrt)
            denom = persist.tile([P, NT], f32)
            nc.vector.tensor_scalar_add(denom[:], norms[:], EPS)
            inv = persist.tile([P, NT], f32)
            nc.vector.reciprocal(inv[:], denom[:])

            out_t = persist.tile([P, 2], f32)
            junk64 = persist.tile([P, NT], f32)
            nc.vector.tensor_mul(junk64[:], gath[:], inv[:])
            nc.vector.tensor_reduce(
                out_t[:, 0:1], junk64[:], mybir.AxisListType.X, ALU.add
            )
            junk64b = persist.tile([P, NT], f32)
            nc.scalar.activation(
                junk64b[:],
                norms[:],
                AF.Square,
                bias=1.0,
                scale=-1.0,
                accum_out=out_t[:, 1:2],
            )
            nc.sync.dma_start(out_d[:], out_t[:])

    nc.compile()
    return nc


def _host_shard(prediction, target):
    """Build per-core input maps."""
    prediction = np.asarray(prediction, dtype=np.float32)
    target = np.asarray(target)

    m128 = (
        (np.arange(SUP * 16)[None, :] % 16) == (np.arange(P)[:, None] % 16)
    ).astype(np.float32)

    in_maps = []
    for k in range(NCORES):
        pred_k = np.ascontiguousarray(prediction[k * R : (k + 1) * R])
        t_k = target[k * R : (k + 1) * R].astype(np.int64)
        # tgt[p, j] = (j % SUP) * C + target[k*R + 128*j + p]
        tk = t_k.reshape(NT, P).T  # [128, 64]
        off = (np.arange(NT) % SUP) * C  # [64]
        tgt_k = (tk + off[None, :]).astype(np.int16)
        in_maps.append({"pred": pred_k, "tgt": tgt_k, "m128": m128})
    return in_maps


def _combine(results):
    """results: list of {'out': [128, 2]} per core -> scalar f32 loss."""
    outs = np.stack([np.asarray(r["out"], dtype=np.float64) for r in results])
    G = outs[:, :, 0].sum()
    NL = outs[:, :, 1].sum()
    loss = -G / N + NORM_FACTOR * (NL / N)
    return np.float32(loss)


def get_nc():
    if "nc" not in _STATE:
        _STATE["nc"] = _build_program()
    return _STATE["nc"]


def kernel(prediction, target):
    from concourse.bass_utils import run_bass_kernel_spmd

    nc = get_nc()
    in_maps = _host_shard(prediction, target)
    res = run_bass_kernel_spmd(nc, in_maps, list(range(NCORES)))
    return _combine(res.results)
